# Initial kernel scaffold
#
"""MoE router kernel for Trainium2 (8 NeuronCores, SPMD).

Problem: x (4,4096,2048) f32, W_gate (64,2048) f32.
  logits = x @ W_gate.T              (16384 tokens x 64 experts)
  top-8 per token -> softmax weights + indices
  aux load-balance loss from per-expert means f (routing fraction) and
  P (mean softmax prob).

Sharding: data-parallel over tokens. 16384 tokens / 8 cores = 2048 each.
W_gate replicated. Per-core partial sums of f and P are combined on host
(the "all-reduce" of the tiny [64] statistics).

Device-side per core:
  - x arrives host-permuted as [p=128, blk=4, kc=16, t=512] so every DMA
    is 128 partitions x 32KB contiguous.
  - gate matmul computed as logitsT = W_chunk.T @ x_chunk accumulating in
    PSUM [64 experts, 512 tokens], then PE-transposed to [128 tok, 64 exp].
  - top-8 via the DVE Max8 / MaxIndex instructions.
  - softmax via ACT exp with per-partition bias and fused accumulation.
  - f / P token-sums via ones-vector matmul (contracts over partitions).
"""

import numpy as np

B, S, D, E = 4, 4096, 2048, 64
TOPK = 8
N_CORES = 8
N_TOK = B * S                  # 16384
T_CORE = N_TOK // N_CORES      # 2048 tokens per core
T_BLK = 512                    # tokens per block
N_BLK = T_CORE // T_BLK        # 4
KCH = D // 128                 # 16 contraction chunks
NGRP = T_BLK // 128            # 4 partition groups per block
LBW = 0.01

# matmul input dtype for the gate projection: "f32r" (1 cyc/row, reduced
# precision path) or "f32" (4 cyc/row, exact fp32 via 2-pass).
MM_DTYPE = "f32"

_RUNNER = None


def _build(mm_dtype=MM_DTYPE):
    import concourse.bass as bass
    import concourse.mybir as mybir
    from concourse.tile import TileContext
    from concourse.masks import make_identity

    f32 = mybir.dt.float32
    u32 = mybir.dt.uint32
    mmdt = mybir.dt.float32r if mm_dtype == "f32r" else mybir.dt.float32

    nc = bass.Bass()
    x_in = nc.declare_dram_parameter("x", [128, N_BLK * KCH * T_BLK], f32,
                                     isOutput=False)
    w_in = nc.declare_dram_parameter("w", [128, KCH * E], f32, isOutput=False)
    w8_out = nc.declare_dram_parameter("w8p", [128, N_BLK * NGRP * TOPK], f32,
                                       isOutput=True)
    ix_out = nc.declare_dram_parameter("ix8p", [128, N_BLK * NGRP * TOPK], u32,
                                       isOutput=True)
    st_out = nc.declare_dram_parameter("st", [2, E], f32, isOutput=True)

    x_v = x_in.rearrange("p (b k t) -> p b k t", b=N_BLK, k=KCH)
    w_v = w_in.rearrange("p (k e) -> p k e", k=KCH)

    with TileContext(nc) as tc:
        with (
            tc.tile_pool(name="singles", bufs=1) as singles,
            tc.tile_pool(name="xpool", bufs=2) as xpool,
            tc.tile_pool(name="work", bufs=2) as work,
            tc.tile_pool(name="small", bufs=2) as small,
            tc.tile_pool(name="pslt", bufs=2, space="PSUM") as pslt,
            tc.tile_pool(name="pstp", bufs=2, space="PSUM") as pstp,
            tc.tile_pool(name="psst", bufs=1, space="PSUM") as psst,
        ):
            # ---- one-time setup ----
            wt_sb = singles.tile([128, KCH, E], f32)
            nc.sync.dma_start(out=wt_sb[:], in_=w_v)
            ones = singles.tile([128, 1], f32)
            nc.vector.memset(ones[:], 1.0)
            ident = singles.tile([64, 64], f32)
            make_identity(nc, ident[:])
            # outputs accumulated in SBUF, one DMA at the end
            wei_all = singles.tile([128, N_BLK, NGRP, TOPK], f32)
            ix_all = singles.tile([128, N_BLK, NGRP, TOPK], u32)
            stats_sb = singles.tile([2, E], f32)

            fstat_ps = psst.tile([1, NGRP * E], f32)
            pstat_ps = psst.tile([1, NGRP * E], f32)

            for b in range(N_BLK):
                # ---- load x block: [128, 16, 512], 4MB contiguous/partition
                x_sb = xpool.tile([128, KCH, T_BLK], f32)
                nc.sync.dma_start(out=x_sb[:], in_=x_v[:, b])

                # ---- gate matmul: logitsT[64 exp, 512 tok] ----
                lt_ps = pslt.tile([E, T_BLK], f32)
                for k in range(KCH):
                    nc.tensor.matmul(
                        lt_ps[:].bitcast(mmdt) if mm_dtype == "f32r" else lt_ps[:],
                        lhsT=wt_sb[:, k, :].bitcast(mmdt),
                        rhs=x_sb[:, k, :].bitcast(mmdt),
                        start=(k == 0), stop=(k == KCH - 1),
                    )
                lt_sb = work.tile([E, T_BLK], f32, tag="lt_sb")
                nc.scalar.copy(lt_sb[:], lt_ps[:])

                # ---- transpose to [128 tok, 64 exp] per 128-token group ----
                tp_ps = pstp.tile([128, NGRP * E], f32)
                for c in range(NGRP):
                    nc.tensor.transpose(
                        tp_ps[:, c * E:(c + 1) * E],
                        lt_sb[:, c * 128:(c + 1) * 128],
                        ident[:],
                    )
                logits = work.tile([128, NGRP, E], f32, tag="logits")
                nc.scalar.copy(logits.rearrange("p c e -> p (c e)"), tp_ps[:])

                # ---- top-8 per token ----
                top8 = small.tile([128, NGRP, TOPK], f32, tag="top8")
                for c in range(NGRP):
                    nc.vector.max(out=top8[:, c, :], in_=logits[:, c, :])
                    nc.vector.max_index(out=ix_all[:, b, c, :],
                                        in_max=top8[:, c, :],
                                        in_values=logits[:, c, :])

                negtop = small.tile([128, NGRP], f32, tag="negtop")
                nc.vector.tensor_scalar(out=negtop[:], in0=top8[:, :, 0],
                                        scalar1=-1.0, scalar2=None,
                                        op0=mybir.AluOpType.mult)

                # ---- softmax over all 64 (for P) and over top-8 (weights) --
                exp64 = work.tile([128, NGRP, E], f32, tag="exp64")
                den = small.tile([128, NGRP], f32, tag="den")
                w8den = small.tile([128, NGRP], f32, tag="w8den")
                maskf = work.tile([128, NGRP, E], f32, tag="maskf")
                for c in range(NGRP):
                    nc.scalar.activation(
                        out=exp64[:, c, :], in_=logits[:, c, :],
                        func=mybir.ActivationFunctionType.Exp,
                        bias=negtop[:, c:c + 1], scale=1.0,
                        accum_out=den[:, c:c + 1])
                    nc.scalar.activation(
                        out=wei_all[:, b, c, :], in_=top8[:, c, :],
                        func=mybir.ActivationFunctionType.Exp,
                        bias=negtop[:, c:c + 1], scale=1.0,
                        accum_out=w8den[:, c:c + 1])
                    # routing mask: logits >= 8th largest
                    nc.vector.tensor_scalar(out=maskf[:, c, :],
                                            in0=logits[:, c, :],
                                            scalar1=top8[:, c, TOPK - 1:TOPK],
                                            scalar2=None,
                                            op0=mybir.AluOpType.is_ge)
                rden = small.tile([128, NGRP], f32, tag="rden")
                rw8 = small.tile([128, NGRP], f32, tag="rw8")
                nc.vector.reciprocal(rden[:], den[:])
                nc.vector.reciprocal(rw8[:], w8den[:])
                for c in range(NGRP):
                    nc.vector.tensor_scalar(out=exp64[:, c, :],
                                            in0=exp64[:, c, :],
                                            scalar1=rden[:, c:c + 1],
                                            scalar2=None,
                                            op0=mybir.AluOpType.mult)
                    nc.vector.tensor_scalar(out=wei_all[:, b, c, :],
                                            in0=wei_all[:, b, c, :],
                                            scalar1=rw8[:, c:c + 1],
                                            scalar2=None,
                                            op0=mybir.AluOpType.mult)

                # ---- token-sum stats via ones-matmul (contract partitions) --
                nc.tensor.matmul(
                    fstat_ps[:],
                    lhsT=ones[:],
                    rhs=maskf.rearrange("p c e -> p (c e)"),
                    start=(b == 0), stop=(b == N_BLK - 1),
                    skip_group_check=True,
                )
                nc.tensor.matmul(
                    pstat_ps[:],
                    lhsT=ones[:],
                    rhs=exp64.rearrange("p c e -> p (c e)"),
                    start=(b == 0), stop=(b == N_BLK - 1),
                    skip_group_check=True,
                )

            # ---- fold the NGRP groups of the stats, write outputs ----
            nc.vector.tensor_reduce(
                out=stats_sb[0:1, :],
                in_=fstat_ps.rearrange("p (c e) -> p e c", e=E),
                op=mybir.AluOpType.add, axis=mybir.AxisListType.X)
            nc.vector.tensor_reduce(
                out=stats_sb[1:2, :],
                in_=pstat_ps.rearrange("p (c e) -> p e c", e=E),
                op=mybir.AluOpType.add, axis=mybir.AxisListType.X)

            nc.sync.dma_start(out=w8_out[:],
                              in_=wei_all.rearrange("p b c k -> p (b c k)"))
            nc.sync.dma_start(out=ix_out[:],
                              in_=ix_all.rearrange("p b c k -> p (b c k)"))
            nc.sync.dma_start(out=st_out[:], in_=stats_sb[:])
    return nc


def _prep_inputs(x, W_gate):
    """Host-side shard + layout permutation.

    Device x layout per core: [p=128, b=4, kc=16, t=512] with
    token = b*512 + (t // 128)*128 ... NOTE: token index inside a block is
    t (0..511); the transpose groups are the 4 sub-chunks of 128 tokens.
    d = kc*128 + p.
    """
    xf = np.ascontiguousarray(x.reshape(N_TOK, D))
    w = np.ascontiguousarray(W_gate)
    # [p, kc, e] layout for the gate weight
    w_dev = np.ascontiguousarray(
        w.T.reshape(KCH, 128, E).transpose(1, 0, 2)).reshape(128, KCH * E)
    in_maps = []
    for i in range(N_CORES):
        xc = xf[i * T_CORE:(i + 1) * T_CORE]          # [2048 tok, 2048 d]
        # -> [b, t, kc, p] -> [p, b, kc, t]
        xd = xc.reshape(N_BLK, T_BLK, KCH, 128).transpose(3, 0, 2, 1)
        xd = np.ascontiguousarray(xd).reshape(128, N_BLK * KCH * T_BLK)
        in_maps.append({"x": xd, "w": w_dev})
    return in_maps


def _assemble(results):
    wlist, ilist = [], []
    fsum = np.zeros(E, np.float64)
    psum = np.zeros(E, np.float64)
    for r in results:
        wp = r["w8p"].reshape(128, N_BLK, NGRP, TOPK)
        ip = r["ix8p"].reshape(128, N_BLK, NGRP, TOPK)
        # token = b*512 + c*128 + p  -> [b, c, p, k]
        wlist.append(wp.transpose(1, 2, 0, 3).reshape(T_CORE, TOPK))
        ilist.append(ip.transpose(1, 2, 0, 3).reshape(T_CORE, TOPK))
        fsum += r["st"][0].astype(np.float64)
        psum += r["st"][1].astype(np.float64)
    weights = np.concatenate(wlist, 0).reshape(B, S, TOPK).astype(np.float32)
    top_idx = np.concatenate(ilist, 0).reshape(B, S, TOPK).astype(np.int32)
    f = fsum / N_TOK
    P = psum / N_TOK
    aux = np.float32(E * float((f * P).sum()) * LBW)
    return weights, top_idx, aux


def _get_runner():
    global _RUNNER
    if _RUNNER is None:
        _RUNNER = _build()
    return _RUNNER


def _run(in_maps, trace=False, **kw):
    from concourse.bass_utils import run_bass_kernel_spmd
    nc = _get_runner()
    return run_bass_kernel_spmd(nc, in_maps, list(range(N_CORES)),
                                trace=trace, **kw)


def kernel(x, W_gate):
    x = np.asarray(x, dtype=np.float32)
    W_gate = np.asarray(W_gate, dtype=np.float32)
    in_maps = _prep_inputs(x, W_gate)
    res = _run(in_maps)
    return _assemble(res.results)


# revision 25
# speedup vs baseline: 2.9035x; 2.9035x over previous
"""MoE router kernel for Trainium2 (8 NeuronCores, SPMD).

Problem: x (4,4096,2048) f32, W_gate (64,2048) f32.
  logits = x @ W_gate.T              (16384 tokens x 64 experts)
  top-8 per token -> softmax weights + indices
  aux load-balance loss from per-expert means f (routing fraction) and
  P (mean softmax prob).

Sharding: data-parallel over tokens. 16384 tokens / 8 cores = 2048 each.
W_gate replicated. Per-core partial sums of f and P are combined on host
(the "all-reduce" of the tiny [64] statistics).

Device-side per core:
  - x arrives host-permuted as [p=128, blk=4, kc=16, t=512] so every DMA
    is 128 partitions x 32KB contiguous.
  - gate matmul computed as logitsT = W_chunk.T @ x_chunk accumulating in
    PSUM [64 experts, 512 tokens], then PE-transposed to [128 tok, 64 exp].
  - top-8 via the DVE Max8 / MaxIndex instructions.
  - softmax via ACT exp with per-partition bias and fused accumulation.
  - f / P token-sums via ones-vector matmul (contracts over partitions).
"""

import numpy as np

B, S, D, E = 4, 4096, 2048, 64
TOPK = 8
N_CORES = 8
N_TOK = B * S                  # 16384
T_CORE = N_TOK // N_CORES      # 2048 tokens per core
T_BLK = 512                    # tokens per block
N_BLK = T_CORE // T_BLK        # 4
KCH = D // 128                 # 16 contraction chunks
NGRP = T_BLK // 128            # 4 partition groups per block
LBW = 0.01

# matmul input dtype for the gate projection: "f32r" (1 cyc/row, reduced
# precision path) or "f32" (4 cyc/row, exact fp32 via 2-pass).
MM_DTYPE = "f32"

_RUNNER = None


def _fix_wait_limits(nc):
    """walrus codegen allows only ONE sync-wait on real engine/DMA
    instructions (Matmult waits ride the single-slot S3_LW struct, DMAs the
    PSEUDO_DMA_DIRECT2D struct). Tile emits up to 2. Hoist surplus waits
    onto a same-engine ENGINE_NOP inserted right before the instruction —
    engines are in-order so this is semantically identical.
    """
    import concourse.mybir as mybir

    SKIP = ("InstEventSemaphore", "InstRegisterMove", "InstCall",
            "InstUnconditionalBranch", "InstISA", "InstConditionalBranch",
            "InstNoOp")
    engines = {e.engine: e for e in
               (nc.tensor, nc.vector, nc.scalar, nc.gpsimd, nc.sync)}
    f = nc.m.functions[0]

    def make_nop(engine):
        # nop() appends to the builder's current (last) block; pop it back out
        # and use it as a standalone carrier instruction.
        bi = engines[engine].nop()
        inst = bi.ins
        for b in f.blocks:
            insts = list(b.instructions)
            names = [i.name for i in insts]
            if inst.name in names:
                insts.pop(names.index(inst.name))
                b.instructions = insts
                break
        return inst

    for blk in f.blocks:
        insts = list(blk.instructions)
        plan = []
        for idx, inst in enumerate(insts):
            t = type(inst).__name__
            si = inst.sync_info
            w = list(si.on_wait) if (si is not None and si.on_wait) else []
            if t not in SKIP and len(w) > 1:
                plan.append((idx, inst, w))
        if not plan:
            continue
        out = []
        plan_idx = {idx: (inst, w) for idx, inst, w in plan}
        for idx, inst in enumerate(insts):
            if idx in plan_idx:
                _, w = plan_idx[idx]
                for surplus in w[:-1]:
                    nop = make_nop(inst.engine)
                    nop.sync_info = mybir.SyncInfo(on_wait=[surplus],
                                                   on_update=[])
                    out.append(nop)
                inst.sync_info.on_wait = w[-1:]
            out.append(inst)
        blk.instructions = out


def _build(mm_dtype=MM_DTYPE, loop_n=1, reps=1, fix_waits=True):
    import concourse.bass as bass
    import concourse.mybir as mybir
    from concourse.tile import TileContext
    from concourse.tile_rust import add_dep_helper
    from concourse.masks import make_identity

    f32 = mybir.dt.float32
    u32 = mybir.dt.uint32
    mmdt = mybir.dt.float32r if mm_dtype == "f32r" else mybir.dt.float32

    nc = bass.Bass()
    x_in = nc.declare_dram_parameter("x", [128, N_BLK * KCH * T_BLK], mmdt,
                                     isOutput=False)
    w_in = nc.declare_dram_parameter("w", [128, KCH * E], mmdt, isOutput=False)
    w8_out = nc.declare_dram_parameter("w8p", [128, N_BLK * NGRP * TOPK], f32,
                                       isOutput=True)
    ix_out = nc.declare_dram_parameter("ix8p", [128, N_BLK * NGRP * TOPK], u32,
                                       isOutput=True)
    st_out = nc.declare_dram_parameter("st", [1, 2 * E], f32, isOutput=True)

    x_v = x_in.rearrange("p (b k t) -> p b k t", b=N_BLK, k=KCH)
    w_v = w_in.rearrange("p (k e) -> p k e", k=KCH)

    with TileContext(nc) as tc:
        with (
            tc.tile_pool(name="singles", bufs=1) as singles,
            tc.tile_pool(name="xpool", bufs=2) as xpool,
            tc.tile_pool(name="work", bufs=2) as work,
            tc.tile_pool(name="small", bufs=2) as small,
            tc.tile_pool(name="pslt", bufs=2, space="PSUM") as pslt,
            tc.tile_pool(name="pstp", bufs=2, space="PSUM") as pstp,
            tc.tile_pool(name="psst", bufs=1, space="PSUM") as psst,
        ):
            # ---- one-time setup ----
            wt_sb = singles.tile([128, KCH, E], mmdt)
            nc.sync.dma_start(out=wt_sb[:], in_=w_v)
            ones = singles.tile([128, 1], f32)
            nc.vector.memset(ones[:], 1.0)
            ident = singles.tile([64, 64], f32)
            make_identity(nc, ident[:])
            # outputs accumulated in SBUF, one DMA at the end
            wei_all = singles.tile([128, N_BLK, NGRP, TOPK], f32)
            ix_all = singles.tile([128, N_BLK, NGRP, TOPK], u32)
            stats_sb = singles.tile([1, 2 * E], f32)

            fstat_ps = psst.tile([1, NGRP * E], f32)
            pstat_ps = psst.tile([1, NGRP * E], f32)

            # Dummy PE consumer of wt_sb: absorbs the weight-DMA semaphore
            # wait so the k-loop matmuls only wait on the x DMA (walrus's
            # LDWEIGHTS slot can carry a single DMA-type wait).
            warm_ps = psst.tile([E, 8], f32)
            nc.tensor.matmul(warm_ps[:], lhsT=wt_sb[:, 0, :],
                             rhs=wt_sb[:, 0, 0:8], start=True, stop=True)

            last_x_reader = {}
            X_BUFS = 2

            def block_body(i, b):
                # ---- load x block: [128, 16, 512], 4MB contiguous/partition
                if i >= X_BUFS:
                    # DMACopy can carry only ONE sync wait in walrus codegen.
                    # Hoist the WAR wait (PE done reading the recycled buffer)
                    # onto a Pool-engine nop so the DMA keeps just the WAW
                    # wait on the previous DMA of this slot.
                    nop = nc.gpsimd.nop()
                    add_dep_helper(nop.ins, last_x_reader[i - X_BUFS].ins,
                                   reason="hoist x WAR wait off DMA")
                x_sb = xpool.tile([128, KCH, T_BLK], mmdt)
                nc.gpsimd.dma_start(out=x_sb[:], in_=x_v[:, b])

                # ---- gate matmul: logitsT[64 exp, 512 tok] ----
                lt_ps = pslt.tile([E, T_BLK], f32)
                for k in range(KCH):
                    mm = nc.tensor.matmul(
                        lt_ps[:],
                        lhsT=wt_sb[:, k, :],
                        rhs=x_sb[:, k, :],
                        start=(k == 0), stop=(k == KCH - 1),
                    )
                last_x_reader[i] = mm
                lt_sb = work.tile([E, T_BLK], f32, tag="lt_sb")
                nc.scalar.copy(lt_sb[:], lt_ps[:])

                # ---- transpose to [128 tok, 64 exp] per 128-token group ----
                tp_ps = pstp.tile([128, NGRP * E], f32)
                for c in range(NGRP):
                    nc.tensor.transpose(
                        tp_ps[:, c * E:(c + 1) * E],
                        lt_sb[:, c * 128:(c + 1) * 128],
                        ident[:],
                    )
                logits = work.tile([128, NGRP, E], f32, tag="logits")
                nc.scalar.copy(logits.rearrange("p c e -> p (c e)"), tp_ps[:])

                # ---- top-8 per token ----
                top8 = small.tile([128, NGRP, TOPK], f32, tag="top8")
                for c in range(NGRP):
                    nc.vector.max(out=top8[:, c, :], in_=logits[:, c, :])
                    nc.vector.max_index(out=ix_all[:, b, c, :],
                                        in_max=top8[:, c, :],
                                        in_values=logits[:, c, :])

                negtop = small.tile([128, NGRP], f32, tag="negtop")
                nc.vector.tensor_scalar(out=negtop[:], in0=top8[:, :, 0],
                                        scalar1=-1.0, scalar2=None,
                                        op0=mybir.AluOpType.mult)

                # ---- softmax over all 64 (for P) and over top-8 (weights) --
                exp64 = work.tile([128, NGRP, E], f32, tag="exp64")
                den = small.tile([128, NGRP], f32, tag="den")
                w8den = small.tile([128, NGRP], f32, tag="w8den")
                maskf = work.tile([128, NGRP, E], f32, tag="maskf")
                for c in range(NGRP):
                    nc.scalar.activation(
                        out=exp64[:, c, :], in_=logits[:, c, :],
                        func=mybir.ActivationFunctionType.Exp,
                        bias=negtop[:, c:c + 1], scale=1.0,
                        accum_out=den[:, c:c + 1])
                    nc.scalar.activation(
                        out=wei_all[:, b, c, :], in_=top8[:, c, :],
                        func=mybir.ActivationFunctionType.Exp,
                        bias=negtop[:, c:c + 1], scale=1.0,
                        accum_out=w8den[:, c:c + 1])
                    # routing mask: logits >= 8th largest
                    nc.vector.tensor_scalar(out=maskf[:, c, :],
                                            in0=logits[:, c, :],
                                            scalar1=top8[:, c, TOPK - 1:TOPK],
                                            scalar2=None,
                                            op0=mybir.AluOpType.is_ge)
                rden = small.tile([128, NGRP], f32, tag="rden")
                rw8 = small.tile([128, NGRP], f32, tag="rw8")
                nc.vector.reciprocal(rden[:], den[:])
                nc.vector.reciprocal(rw8[:], w8den[:])
                for c in range(NGRP):
                    nc.vector.tensor_scalar(out=exp64[:, c, :],
                                            in0=exp64[:, c, :],
                                            scalar1=rden[:, c:c + 1],
                                            scalar2=None,
                                            op0=mybir.AluOpType.mult)
                    nc.vector.tensor_scalar(out=wei_all[:, b, c, :],
                                            in0=wei_all[:, b, c, :],
                                            scalar1=rw8[:, c:c + 1],
                                            scalar2=None,
                                            op0=mybir.AluOpType.mult)

                # ---- token-sum stats via ones-matmul (contract partitions) --
                nc.tensor.matmul(
                    fstat_ps[:],
                    lhsT=ones[:],
                    rhs=maskf.rearrange("p c e -> p (c e)"),
                    start=(b == 0), stop=(b == N_BLK - 1),
                    skip_group_check=True,
                )
                nc.tensor.matmul(
                    pstat_ps[:],
                    lhsT=ones[:],
                    rhs=exp64.rearrange("p c e -> p (c e)"),
                    start=(b == 0), stop=(b == N_BLK - 1),
                    skip_group_check=True,
                )

            def iteration(it):
                for j in range(N_BLK):
                    block_body(it * N_BLK + j, j)

            if loop_n > 1:
                # NOTE: For_i loops fail walrus codegen in this container
                # ("ISA wrong length" on the loop sem-reset path); kept only
                # for experiments.
                with tc.For_i(0, loop_n, 1):
                    iteration(0)
            else:
                for it in range(reps):
                    iteration(it)

            # ---- fold the NGRP groups of the stats, write outputs ----
            nc.vector.tensor_reduce(
                out=stats_sb[0:1, 0:E],
                in_=fstat_ps.rearrange("p (c e) -> p e c", e=E),
                op=mybir.AluOpType.add, axis=mybir.AxisListType.X)
            nc.vector.tensor_reduce(
                out=stats_sb[0:1, E:2 * E],
                in_=pstat_ps.rearrange("p (c e) -> p e c", e=E),
                op=mybir.AluOpType.add, axis=mybir.AxisListType.X)

            nc.sync.dma_start(out=w8_out[:],
                              in_=wei_all.rearrange("p b c k -> p (b c k)"))
            nc.sync.dma_start(out=ix_out[:],
                              in_=ix_all.rearrange("p b c k -> p (b c k)"))
            nc.sync.dma_start(out=st_out[:], in_=stats_sb[:])
    if fix_waits:
        _fix_wait_limits(nc)
    return nc


def _prep_inputs(x, W_gate):
    """Host-side shard + layout permutation.

    Device x layout per core: [p=128, b=4, kc=16, t=512] with
    token = b*512 + (t // 128)*128 ... NOTE: token index inside a block is
    t (0..511); the transpose groups are the 4 sub-chunks of 128 tokens.
    d = kc*128 + p.
    """
    xf = np.ascontiguousarray(x.reshape(N_TOK, D))
    w = np.ascontiguousarray(W_gate)
    # [p, kc, e] layout for the gate weight
    w_dev = np.ascontiguousarray(
        w.T.reshape(KCH, 128, E).transpose(1, 0, 2)).reshape(128, KCH * E)
    in_maps = []
    for i in range(N_CORES):
        xc = xf[i * T_CORE:(i + 1) * T_CORE]          # [2048 tok, 2048 d]
        # -> [b, t, kc, p] -> [p, b, kc, t]
        xd = xc.reshape(N_BLK, T_BLK, KCH, 128).transpose(3, 0, 2, 1)
        xd = np.ascontiguousarray(xd).reshape(128, N_BLK * KCH * T_BLK)
        in_maps.append({"x": xd, "w": w_dev})
    return in_maps


def _assemble(results):
    wlist, ilist = [], []
    fsum = np.zeros(E, np.float64)
    psum = np.zeros(E, np.float64)
    for r in results:
        wp = r["w8p"].reshape(128, N_BLK, NGRP, TOPK)
        ip = r["ix8p"].reshape(128, N_BLK, NGRP, TOPK)
        # token = b*512 + c*128 + p  -> [b, c, p, k]
        wlist.append(wp.transpose(1, 2, 0, 3).reshape(T_CORE, TOPK))
        ilist.append(ip.transpose(1, 2, 0, 3).reshape(T_CORE, TOPK))
        fsum += r["st"][0, :E].astype(np.float64)
        psum += r["st"][0, E:].astype(np.float64)
    weights = np.concatenate(wlist, 0).reshape(B, S, TOPK).astype(np.float32)
    top_idx = np.concatenate(ilist, 0).reshape(B, S, TOPK).astype(np.int32)
    f = fsum / N_TOK
    P = psum / N_TOK
    aux = np.float32(E * float((f * P).sum()) * LBW)
    return weights, top_idx, aux


def _get_runner():
    global _RUNNER
    if _RUNNER is None:
        _RUNNER = _build()
    return _RUNNER


def _run(in_maps, trace=False, **kw):
    from concourse.bass_utils import run_bass_kernel_spmd
    nc = _get_runner()
    return run_bass_kernel_spmd(nc, in_maps, list(range(N_CORES)),
                                trace=trace, **kw)


def kernel(x, W_gate):
    x = np.asarray(x, dtype=np.float32)
    W_gate = np.asarray(W_gate, dtype=np.float32)
    in_maps = _prep_inputs(x, W_gate)
    res = _run(in_maps)
    return _assemble(res.results)
